# revision 24
# baseline (speedup 1.0000x reference)
"""BCH/RS systematic encoder kernel for Trainium2 (8 NeuronCores, data parallel).

Computes out = concat([msg, (msg @ Gp) mod 2], axis=-1) for
msg [16384, 1000] f32 of 0/1 bits and Gp [1000, 256] f32 of 0/1 bits.

Design v12 (per core, 2048 rows, 16 pipeline units of 128 rows):
  - msg is 0/1 bits, so the host shards it to the device as fp8e4 (exact,
    same as the host-side Gp swizzle), pre-padded to 1024 k and pre-swizzled
    to partition-major [128, 16*1024] so the load is one contiguous 16KB run
    per partition. Per-core HBM traffic drops to 2.1 MB read + 10.29 MB f32
    write (the output write is the floor).
  - The device upcasts fp8 -> f32 for the copy-through columns (exact,
    column-split between ACT and DVE) straight into the f32 output-row
    tiles, and the PE transposes the fp8 blocks directly (no cast step).
  - PE transposes plain fp8 [128,128] blocks (nc.tensor.transpose against a
    host-loaded fp8 identity) into PSUM; the fp8 transpose datapath writes
    one value per 16-bit PSUM lane (ISA "output element step of 2"); ACT
    gathers the even bytes back to SBUF. (Tile serializes xbar-transpose
    DMAs against ALL concurrent DMAs, so no DMA transposes anywhere.)
  - DoubleRow fp8 matmuls: two adjacent transposed blocks form the
    [128, 2, 128] block-layout weights AP, contracting k = 256g + 128i + q
    against host-swizzled Gp rows; f32 PSUM accumulation is exact.
  - DVE evicts parity PSUM f32 -> i32, ANDs with 1 (mod 2), copies i32 -> f32
    into the output-row tile; SWDGE stores finished f32 rows on their own
    queue so stores interleave with the (small) loads from the start.
"""

import os
import sys

import numpy as np

if os.path.isdir("/opt/trn_rl_repo") and "/opt/trn_rl_repo" not in sys.path:
    sys.path.insert(0, "/opt/trn_rl_repo")

import ml_dtypes

import concourse.bacc as bacc
import concourse.mybir as mybir
import concourse.tile as tile
from concourse.bass_utils import run_bass_kernel_spmd

BATCH = 16384
MSG = 1000
NPAR = 256
NCORES = 8
ROWS = BATCH // NCORES  # 2048
P = 128
KB = 4  # k pair-blocks of 256; padded K = 1024
KPAD = KB * 2 * P

# test.py pokes these for profiling
TRACE = False
LAST_RESULT = None

_CACHE = {}

F8 = mybir.dt.float8e4


def build_nc(rows=ROWS):
    """Emit the Bass/Tile IR for one core handling `rows` rows."""
    n_units = rows // P
    nc = bacc.Bacc("TRN2", target_bir_lowering=False, debug=False)
    msgf8 = nc.dram_tensor(
        "msgf8", [P, n_units * KPAD], F8, kind="ExternalInput"
    )
    gp = nc.dram_tensor("gp", [P, KB, 2, NPAR], F8, kind="ExternalInput")
    ident = nc.dram_tensor("ident", [P, P], F8, kind="ExternalInput")
    out = nc.dram_tensor(
        "out", [rows, MSG + NPAR], mybir.dt.float32, kind="ExternalOutput"
    )

    out2 = out[:, :].rearrange("(s p) k -> s p k", p=P)

    # load split: first pieces small so unit 0 starts early
    piece_ends = sorted({u for u in (1, 2, 4, 8, 12, n_units) if u <= n_units})

    with tile.TileContext(nc) as tc:
        with (
            tc.tile_pool(name="gpool", bufs=1) as gpool,
            tc.tile_pool(name="opool", bufs=n_units) as opool,
            tc.tile_pool(name="tpool", bufs=2) as tpool,
            tc.tile_pool(name="cpool", bufs=2) as cpool,
            tc.tile_pool(name="epool", bufs=2) as epool,
            tc.tile_pool(name="tppool", bufs=2, space="PSUM") as tppool,
            tc.tile_pool(name="ppool", bufs=2, space="PSUM") as ppool,
        ):
            # Gp resident in SBUF: gsb[q, g, i, n] = Gp_padded[256g + 128i + q, n]
            gsb = gpool.tile([P, KB, 2, NPAR], F8)
            nc.sync.dma_start(out=gsb[:, :, :, :], in_=gp[:, :, :, :])
            idsb = gpool.tile([P, P], F8)
            nc.sync.dma_start(out=idsb[:, :], in_=ident[:, :])

            # whole core's fp8 msg resident (16KB/partition), loaded on the
            # sync ring in a few contiguous pieces (small first for fast start)
            f8all = gpool.tile([P, n_units, KPAD], F8)
            prev = 0
            for u in piece_ends:
                nc.sync.dma_start(
                    out=f8all[:, prev:u, :],
                    in_=msgf8[:, prev * KPAD : u * KPAD],
                )
                prev = u

            otiles, tps, ts, accs = {}, {}, {}, {}

            def emit_upcast(si):
                # copy-through columns fp8 -> f32 (exact), split ACT/DVE.
                # ACT gets the smaller share since it also runs the evicts
                # (ACT/unit ~= 0.49 + 1.05 vs DVE ~= 0.91 + 0.57 -- balanced)
                o = opool.tile([P, MSG + NPAR], mybir.dt.float32, tag="o")
                nc.scalar.copy(o[:, 0:352], f8all[:, si, 0:352])
                nc.vector.tensor_copy(o[:, 352:MSG], f8all[:, si, 352:MSG])
                otiles[si] = o

            def emit_transpose(si):
                # PE transpose of plain fp8 blocks -> PSUM; output element
                # step 2 (one fp8 per 16-bit PSUM lane)
                tp = tppool.tile([P, 8, 2 * P], F8, tag="tp")
                for blk in range(8):
                    nc.tensor.transpose(
                        tp[:, blk, :].rearrange("q (m two) -> q m two", two=2)[
                            :, :, 0
                        ],
                        f8all[:, si, 128 * blk : 128 * (blk + 1)],
                        idsb[:, :],
                    )
                tps[si] = tp

            def emit_evict(si):
                # transposed blocks PSUM -> SBUF on ACT (gather even bytes)
                t = tpool.tile([P, 8, P], F8, tag="t")
                nc.scalar.copy(
                    t[:, :, :],
                    tps.pop(si)[:, :, :].rearrange(
                        "q s (m two) -> q s m two", two=2
                    )[:, :, :, 0],
                )
                ts[si] = t

            def emit_matmul(si):
                # DoubleRow fp8 matmuls over adjacent transposed block pairs
                t = ts.pop(si)
                acc = ppool.tile([P, NPAR], mybir.dt.float32, tag="acc")
                for g in range(KB):
                    nc.tensor.matmul(
                        acc[:, :],
                        t[:, 2 * g : 2 * g + 2, :],
                        gsb[:, g, :, :],
                        start=(g == 0),
                        stop=(g == KB - 1),
                        perf_mode=mybir.MatmulPerfMode.DoubleRow,
                    )
                accs[si] = acc

            def emit_parity_store(si):
                o = otiles.pop(si)
                # exact-integer f32 -> i32 eviction, mod 2 == AND 1, parity
                # i32 -> f32 into the output-row tile: all DVE
                ci = cpool.tile([P, NPAR], mybir.dt.int32, tag="ci")
                nc.vector.tensor_copy(ci[:, :], accs.pop(si)[:, :])
                e = epool.tile([P, NPAR], mybir.dt.int32, tag="e")
                nc.vector.tensor_scalar(
                    e[:, :], ci[:, :], 1, None, mybir.AluOpType.bitwise_and
                )
                nc.vector.tensor_copy(o[:, MSG : MSG + NPAR], e[:, :])
                # plain f32 store via SWDGE: its own engine stream and queue
                nc.gpsimd.dma_start(out=out2[si, :, :], in_=o[:, :])

            # software-pipelined emission: per engine stream, everything a
            # unit's store needs (evict -> mm -> parity) is emitted before the
            # NEXT unit's work, and transpose(si+1) lands before matmul(si)
            # on the PE stream
            emit_upcast(0)
            emit_transpose(0)
            for si in range(n_units):
                emit_evict(si)
                emit_matmul(si)
                emit_parity_store(si)
                if si + 1 < n_units:
                    emit_upcast(si + 1)
                    emit_transpose(si + 1)

    nc.compile()
    return nc


def prep_gp(Gp):
    """Pad Gp to 1024 rows and swizzle to [128, 4, 2, 256] fp8:
    gsw[q, g, i, n] = Gp_pad[256*g + 128*i + q, n]
    """
    gp = np.asarray(Gp, dtype=np.float32)
    gp_pad = np.zeros((KPAD, NPAR), dtype=np.float32)
    gp_pad[:MSG] = gp
    gsw = gp_pad.reshape(KB, 2, P, NPAR).transpose(2, 0, 1, 3)
    return np.ascontiguousarray(gsw).astype(ml_dtypes.float8_e4m3)


def prep_msg(msg):
    """Cast 0/1 f32 message bits to fp8 (exact), pad k to 1024, and swizzle
    each core's slice to partition-major [128, n_units*1024]:
    row s*128 + p -> partition p, unit s."""
    f8 = np.zeros((BATCH, KPAD), dtype=ml_dtypes.float8_e4m3)
    f8[:, :MSG] = msg.astype(ml_dtypes.float8_e4m3)
    n_units = ROWS // P
    per_core = []
    for i in range(NCORES):
        sl = f8[i * ROWS : (i + 1) * ROWS]
        sw = sl.reshape(n_units, P, KPAD).transpose(1, 0, 2).reshape(P, -1)
        per_core.append(np.ascontiguousarray(sw))
    return per_core


def kernel(message_bits, Gp):
    global LAST_RESULT
    msg = np.ascontiguousarray(np.asarray(message_bits, dtype=np.float32))
    assert msg.shape == (BATCH, MSG), msg.shape
    gsw = prep_gp(Gp)
    ident = np.eye(P, dtype=np.float32).astype(ml_dtypes.float8_e4m3)
    msg_cores = prep_msg(msg)

    if "nc" not in _CACHE:
        _CACHE["nc"] = build_nc()
    nc = _CACHE["nc"]

    in_maps = [
        {"msgf8": msg_cores[i], "gp": gsw, "ident": ident}
        for i in range(NCORES)
    ]
    res = run_bass_kernel_spmd(
        nc, in_maps, core_ids=list(range(NCORES)), trace=TRACE
    )
    LAST_RESULT = res
    return np.concatenate([r["out"] for r in res.results], axis=0)


# revision 25
# speedup vs baseline: 1.0188x; 1.0188x over previous
"""BCH/RS systematic encoder kernel for Trainium2 (8 NeuronCores, data parallel).

Computes out = concat([msg, (msg @ Gp) mod 2], axis=-1) for
msg [16384, 1000] f32 of 0/1 bits and Gp [1000, 256] f32 of 0/1 bits.

Design v12 (per core, 2048 rows, 16 pipeline units of 128 rows):
  - msg is 0/1 bits, so the host shards it to the device as fp8e4 (exact,
    same as the host-side Gp swizzle), pre-padded to 1024 k and pre-swizzled
    to partition-major [128, 16*1024] so the load is one contiguous 16KB run
    per partition. Per-core HBM traffic drops to 2.1 MB read + 10.29 MB f32
    write (the output write is the floor).
  - The device upcasts fp8 -> f32 for the copy-through columns (exact,
    column-split between ACT and DVE) straight into the f32 output-row
    tiles, and the PE transposes the fp8 blocks directly (no cast step).
  - PE transposes plain fp8 [128,128] blocks (nc.tensor.transpose against a
    host-loaded fp8 identity) into PSUM; the fp8 transpose datapath writes
    one value per 16-bit PSUM lane (ISA "output element step of 2"); ACT
    gathers the even bytes back to SBUF. (Tile serializes xbar-transpose
    DMAs against ALL concurrent DMAs, so no DMA transposes anywhere.)
  - DoubleRow fp8 matmuls: two adjacent transposed blocks form the
    [128, 2, 128] block-layout weights AP, contracting k = 256g + 128i + q
    against host-swizzled Gp rows; f32 PSUM accumulation is exact.
  - DVE evicts parity PSUM f32 -> i32, ANDs with 1 (mod 2), copies i32 -> f32
    into the output-row tile; SWDGE stores finished f32 rows on their own
    queue so stores interleave with the (small) loads from the start.
"""

import os
import sys

import numpy as np

if os.path.isdir("/opt/trn_rl_repo") and "/opt/trn_rl_repo" not in sys.path:
    sys.path.insert(0, "/opt/trn_rl_repo")

import ml_dtypes

import concourse.bacc as bacc
import concourse.mybir as mybir
import concourse.tile as tile
from concourse.bass_utils import run_bass_kernel_spmd

BATCH = 16384
MSG = 1000
NPAR = 256
NCORES = 8
ROWS = BATCH // NCORES  # 2048
P = 128
KB = 4  # k pair-blocks of 256; padded K = 1024
KPAD = KB * 2 * P

# test.py pokes these for profiling
TRACE = False
LAST_RESULT = None

_CACHE = {}

F8 = mybir.dt.float8e4


def build_nc(rows=ROWS):
    """Emit the Bass/Tile IR for one core handling `rows` rows."""
    n_units = rows // P
    nc = bacc.Bacc("TRN2", target_bir_lowering=False, debug=False)
    msgf8 = nc.dram_tensor(
        "msgf8", [P, n_units * KPAD], F8, kind="ExternalInput"
    )
    gp = nc.dram_tensor("gp", [P, KB, 2, NPAR], F8, kind="ExternalInput")
    ident = nc.dram_tensor("ident", [P, P], F8, kind="ExternalInput")
    out = nc.dram_tensor(
        "out", [rows, MSG + NPAR], mybir.dt.float32, kind="ExternalOutput"
    )

    out2 = out[:, :].rearrange("(s p) k -> s p k", p=P)

    # load split: first pieces small so unit 0 starts early
    piece_ends = sorted({u for u in (1, 2, 4, 8, 12, n_units) if u <= n_units})

    with tile.TileContext(nc) as tc:
        with (
            tc.tile_pool(name="gpool", bufs=1) as gpool,
            tc.tile_pool(name="opool", bufs=n_units) as opool,
            tc.tile_pool(name="tpool", bufs=2) as tpool,
            tc.tile_pool(name="cpool", bufs=2) as cpool,
            tc.tile_pool(name="epool", bufs=2) as epool,
            tc.tile_pool(name="tppool", bufs=2, space="PSUM") as tppool,
            tc.tile_pool(name="ppool", bufs=2, space="PSUM") as ppool,
        ):
            # Gp resident in SBUF: gsb[q, g, i, n] = Gp_padded[256g + 128i + q, n]
            gsb = gpool.tile([P, KB, 2, NPAR], F8)
            nc.sync.dma_start(out=gsb[:, :, :, :], in_=gp[:, :, :, :])
            idsb = gpool.tile([P, P], F8)
            nc.sync.dma_start(out=idsb[:, :], in_=ident[:, :])

            # whole core's fp8 msg resident (16KB/partition), loaded on the
            # sync ring in a few contiguous pieces (small first for fast start)
            f8all = gpool.tile([P, n_units, KPAD], F8)
            prev = 0
            for u in piece_ends:
                nc.sync.dma_start(
                    out=f8all[:, prev:u, :],
                    in_=msgf8[:, prev * KPAD : u * KPAD],
                )
                prev = u

            otiles, tps, ts, accs = {}, {}, {}, {}

            def emit_upcast(si):
                # copy-through columns fp8 -> f32 (exact), split ACT/DVE
                # evenly (measured best: ACT is faster per element, so giving
                # DVE more columns to "balance" the evicts regresses)
                o = opool.tile([P, MSG + NPAR], mybir.dt.float32, tag="o")
                nc.scalar.copy(o[:, 0:500], f8all[:, si, 0:500])
                nc.vector.tensor_copy(o[:, 500:MSG], f8all[:, si, 500:MSG])
                otiles[si] = o

            def emit_transpose(si):
                # PE transpose of plain fp8 blocks -> PSUM; output element
                # step 2 (one fp8 per 16-bit PSUM lane)
                tp = tppool.tile([P, 8, 2 * P], F8, tag="tp")
                for blk in range(8):
                    nc.tensor.transpose(
                        tp[:, blk, :].rearrange("q (m two) -> q m two", two=2)[
                            :, :, 0
                        ],
                        f8all[:, si, 128 * blk : 128 * (blk + 1)],
                        idsb[:, :],
                    )
                tps[si] = tp

            def emit_evict(si):
                # transposed blocks PSUM -> SBUF on ACT (gather even bytes)
                t = tpool.tile([P, 8, P], F8, tag="t")
                nc.scalar.copy(
                    t[:, :, :],
                    tps.pop(si)[:, :, :].rearrange(
                        "q s (m two) -> q s m two", two=2
                    )[:, :, :, 0],
                )
                ts[si] = t

            def emit_matmul(si):
                # DoubleRow fp8 matmuls over adjacent transposed block pairs
                t = ts.pop(si)
                acc = ppool.tile([P, NPAR], mybir.dt.float32, tag="acc")
                for g in range(KB):
                    nc.tensor.matmul(
                        acc[:, :],
                        t[:, 2 * g : 2 * g + 2, :],
                        gsb[:, g, :, :],
                        start=(g == 0),
                        stop=(g == KB - 1),
                        perf_mode=mybir.MatmulPerfMode.DoubleRow,
                    )
                accs[si] = acc

            def emit_parity_store(si):
                o = otiles.pop(si)
                # exact-integer f32 -> i32 eviction, mod 2 == AND 1, parity
                # i32 -> f32 into the output-row tile: all DVE
                ci = cpool.tile([P, NPAR], mybir.dt.int32, tag="ci")
                nc.vector.tensor_copy(ci[:, :], accs.pop(si)[:, :])
                e = epool.tile([P, NPAR], mybir.dt.int32, tag="e")
                nc.vector.tensor_scalar(
                    e[:, :], ci[:, :], 1, None, mybir.AluOpType.bitwise_and
                )
                nc.vector.tensor_copy(o[:, MSG : MSG + NPAR], e[:, :])
                # plain f32 store via SWDGE: its own engine stream and queue
                nc.gpsimd.dma_start(out=out2[si, :, :], in_=o[:, :])

            # software-pipelined emission: per engine stream, everything a
            # unit's store needs (evict -> mm -> parity) is emitted before the
            # NEXT unit's work, and transpose(si+1) lands before matmul(si)
            # on the PE stream
            emit_upcast(0)
            emit_transpose(0)
            for si in range(n_units):
                emit_evict(si)
                emit_matmul(si)
                emit_parity_store(si)
                if si + 1 < n_units:
                    emit_upcast(si + 1)
                    emit_transpose(si + 1)

    nc.compile()
    return nc


def prep_gp(Gp):
    """Pad Gp to 1024 rows and swizzle to [128, 4, 2, 256] fp8:
    gsw[q, g, i, n] = Gp_pad[256*g + 128*i + q, n]
    """
    gp = np.asarray(Gp, dtype=np.float32)
    gp_pad = np.zeros((KPAD, NPAR), dtype=np.float32)
    gp_pad[:MSG] = gp
    gsw = gp_pad.reshape(KB, 2, P, NPAR).transpose(2, 0, 1, 3)
    return np.ascontiguousarray(gsw).astype(ml_dtypes.float8_e4m3)


def prep_msg(msg):
    """Cast 0/1 f32 message bits to fp8 (exact), pad k to 1024, and swizzle
    each core's slice to partition-major [128, n_units*1024]:
    row s*128 + p -> partition p, unit s."""
    f8 = np.zeros((BATCH, KPAD), dtype=ml_dtypes.float8_e4m3)
    f8[:, :MSG] = msg.astype(ml_dtypes.float8_e4m3)
    n_units = ROWS // P
    per_core = []
    for i in range(NCORES):
        sl = f8[i * ROWS : (i + 1) * ROWS]
        sw = sl.reshape(n_units, P, KPAD).transpose(1, 0, 2).reshape(P, -1)
        per_core.append(np.ascontiguousarray(sw))
    return per_core


def kernel(message_bits, Gp):
    global LAST_RESULT
    msg = np.ascontiguousarray(np.asarray(message_bits, dtype=np.float32))
    assert msg.shape == (BATCH, MSG), msg.shape
    gsw = prep_gp(Gp)
    ident = np.eye(P, dtype=np.float32).astype(ml_dtypes.float8_e4m3)
    msg_cores = prep_msg(msg)

    if "nc" not in _CACHE:
        _CACHE["nc"] = build_nc()
    nc = _CACHE["nc"]

    in_maps = [
        {"msgf8": msg_cores[i], "gp": gsw, "ident": ident}
        for i in range(NCORES)
    ]
    res = run_bass_kernel_spmd(
        nc, in_maps, core_ids=list(range(NCORES)), trace=TRACE
    )
    LAST_RESULT = res
    return np.concatenate([r["out"] for r in res.results], axis=0)
